# revision 20
# baseline (speedup 1.0000x reference)
"""Trainium2 Bass kernel for nn_MHA_2516850835986.

MHA: B=1, T=2048, C=2048, H=32 heads, d=64, causal, RoPE (head-indexed
angle quirk: within head h all feature pairs rotate by t * 10000^(-h/32)).

Sharding: head-parallel across 8 cores (4 heads each). x is replicated
(pre-transposed on host), qkv columns / proj rows sharded by head. Each
core produces a partial [T, C] output (proj contraction over its own
heads' features); partials are summed on host.

All matmul operands are bf16 (1 cyc/row streaming AND fast LDWEIGHTS so
16-chunk accumulation chains are not load-gated). PSUM stays f32.

Schedule is software-pipelined around the exp-bound attention loop: the
scalar engine's exp (~1.1us per [128,2,512] s-block pair) is the
attention rate limiter, so the PE stream for tile i's attention is
padded with "filler" work — tile i+1's qkv/v/rope chunk matmuls and
tile i-1's proj matmuls — keeping the PE at 100% and at full p-state.
Scores/av/exp on the diagonal 512-tile are column-restricted (only
t >= 128*b computed per s-subblock), so no masked-region memsets are
needed and only the exact [128,128] diagonal band gets a keep-mask
multiply (gpsimd).
"""

import sys

sys.path.insert(0, "/opt/trn_rl_repo")
import numpy as np

T = 2048
C = 2048
NH = 32          # total heads
HL = 4           # heads per core
D = 64           # head dim
NC_ = 8          # cores
TT = 512         # t-tile width
NTT = T // TT    # 4 t-tiles
KC = C // 128    # 16 contraction chunks
ROPE_THETA = 10000.0

_CACHE = {}


def _build_program():
    import concourse.bass as bass
    import concourse.tile as tile
    from concourse import bacc, mybir
    from contextlib import ExitStack

    F32 = mybir.dt.float32
    F32R = mybir.dt.float32r
    BF16 = mybir.dt.bfloat16
    EXP = mybir.ActivationFunctionType.Exp
    MUL = mybir.AluOpType.mult
    ADD = mybir.AluOpType.add

    nc = bacc.Bacc(None, target_bir_lowering=False)

    xt = nc.declare_dram_parameter("xt", [C, T], BF16, False)          # x^T
    wqk = nc.declare_dram_parameter("wqk", [C, 4 * 128], BF16, False)  # q|k cols
    wv = nc.declare_dram_parameter("wv", [C, 256], BF16, False)
    wproj = nc.declare_dram_parameter("wproj", [256, T], BF16, False)
    costab = nc.declare_dram_parameter("costab", [128, 2, T], BF16, False)
    sintab = nc.declare_dram_parameter("sintab", [128, 2, T], BF16, False)
    tri = nc.declare_dram_parameter("tri", [128, 128], BF16, False)    # diag band keep-mask
    perm = nc.declare_dram_parameter("perm", [128, 128], BF16, False)   # pair-swap
    out = nc.declare_dram_parameter("out", [T, T], BF16, True)

    xt_v = xt.rearrange("(kc p) t -> p kc t", p=128)
    wqk_v = wqk.rearrange("(kc p) m -> p kc m", p=128)
    wv_v = wv.rearrange("(kc p) m -> p kc m", p=128)
    wproj_v = wproj.rearrange("(b p) n -> p b n", p=128)

    with tile.TileContext(nc) as tc, ExitStack() as ctx:
        consts = ctx.enter_context(tc.tile_pool(name="consts", bufs=1))
        xtp = ctx.enter_context(tc.tile_pool(name="xtp", bufs=3))
        csp = ctx.enter_context(tc.tile_pool(name="csp", bufs=3))
        qrawp = ctx.enter_context(tc.tile_pool(name="qrawp", bufs=1))
        qrotp = ctx.enter_context(tc.tile_pool(name="qrotp", bufs=2))
        persist = ctx.enter_context(tc.tile_pool(name="persist", bufs=1))
        p4p = ctx.enter_context(tc.tile_pool(name="p4p", bufs=3))
        ytp = ctx.enter_context(tc.tile_pool(name="ytp", bufs=2))
        ytmpp = ctx.enter_context(tc.tile_pool(name="ytmpp", bufs=2))
        ymp = ctx.enter_context(tc.tile_pool(name="ymp", bufs=4))
        rp = ctx.enter_context(tc.tile_pool(name="rp", bufs=2))
        outp = ctx.enter_context(tc.tile_pool(name="outp", bufs=2))

        # PSUM (8 banks): S pairs 2x2 + psy A/B 1x2 + everything else 1x2
        sps = ctx.enter_context(tc.tile_pool(name="sps", bufs=2, space="PSUM"))
        psyp = ctx.enter_context(tc.tile_pool(name="psyp", bufs=2, space="PSUM"))
        unips = ctx.enter_context(tc.tile_pool(name="unips", bufs=2, space="PSUM"))

        wqk_sb = consts.tile([128, KC, 512], BF16)
        wv_sb = consts.tile([128, KC, 256], BF16)
        wproj_sb = consts.tile([128, 2, T], BF16)
        tri_sb = consts.tile([128, 128], BF16)
        perm_sb = consts.tile([128, 128], BF16)
        ones_sb = consts.tile([1, 64], BF16)
        nc.vector.memset(ones_sb[:], 1.0)

        # v in normal layout [s, dd]: per s-block slot of 4 heads x (64 v + 1 one + 1 pad)
        v_sb = persist.tile([128, KC, HL, 66], BF16)
        nc.vector.memset(v_sb[:].rearrange("p a b c -> p (a b c)"), 1.0)
        # k^T (rope'd), persistent across tiles: [dd(2 heads), block, t]
        krot = persist.tile([128, 2, T], BF16)

        loads = {}

        def load_tile(j):
            """Issue input DMAs for t-tile j (sync HWDGE queue only)."""
            tslj = slice(TT * j, TT * (j + 1))
            xth = []
            for half in range(2):
                xh = xtp.tile([128, KC // 2, TT], BF16, tag="xt")
                nc.sync.dma_start(xh[:], xt_v[:, (KC // 2) * half:(KC // 2) * (half + 1), tslj])
                xth.append(xh)
            cos_t = csp.tile([128, 2, TT], BF16, tag="cos")
            nc.sync.dma_start(cos_t[:], costab[:, :, tslj])
            sin_t = csp.tile([128, 2, TT], BF16, tag="sin")
            nc.sync.dma_start(sin_t[:], sintab[:, :, tslj])
            loads[j] = (xth, cos_t, sin_t)

        # ---- preamble: tile-0 inputs interleaved with the constants in
        # quarter chunks so the first qk chain starts early ----
        xh0 = xtp.tile([128, KC // 2, TT], BF16, tag="xt")
        xh1 = xtp.tile([128, KC // 2, TT], BF16, tag="xt")
        xq = [xh0[:, 0:4, :], xh0[:, 4:8, :], xh1[:, 0:4, :], xh1[:, 4:8, :]]
        for q in range(4):
            nc.sync.dma_start(wqk_sb[:, 4 * q:4 * (q + 1), :],
                              wqk_v[:, 4 * q:4 * (q + 1), :])
            nc.sync.dma_start(xq[q], xt_v[:, 4 * q:4 * (q + 1), 0:TT])
        cos0 = csp.tile([128, 2, TT], BF16, tag="cos")
        nc.sync.dma_start(cos0[:], costab[:, :, 0:TT])
        sin0 = csp.tile([128, 2, TT], BF16, tag="sin")
        nc.sync.dma_start(sin0[:], sintab[:, :, 0:TT])
        nc.sync.dma_start(wv_sb[:], wv_v[:])
        nc.sync.dma_start(perm_sb[:], perm[:])
        nc.sync.dma_start(tri_sb[:], tri[:])
        nc.sync.dma_start(wproj_sb[:], wproj_v[:])
        loads[0] = ([xh0, xh1], cos0, sin0)

        # ---------- filler machinery ----------
        qfront = []   # (est_ns, closure) for tile i+1 qkv/v/rope
        qproj = []    # (est_ns, closure) for tile i-1 proj
        mode = {"attn": True}   # in-attention drains keep copies off scalar

        def drain(budget_ns):
            while budget_ns > 0 and (qproj or qfront):
                est, fn = (qproj or qfront).pop(0)
                fn()
                budget_ns -= est

        def drain_front_all():
            mode["attn"] = False
            while qfront:
                _, fn = qfront.pop(0)
                fn()
            mode["attn"] = True

        def drain_all():
            mode["attn"] = False
            while qproj or qfront:
                _, fn = (qproj or qfront).pop(0)
                fn()
            mode["attn"] = True

        def make_front_units(j):
            """qkv/v/rope chunk-matmul closures for t-tile j."""
            xth, cos_t, sin_t = loads[j]
            tslj = slice(TT * j, TT * (j + 1))
            qraw = qrawp.tile([128, 4, TT], BF16, tag="qraw")
            qrot = qrotp.tile([128, 2, TT], BF16, tag="qrot")
            units = []
            if j + 1 < NTT:
                units.append((50, lambda: load_tile(j + 1)))
            st = {}

            def qk_chunk(m, c4, st=st):
                if c4 == 0:
                    st[m] = unips.tile([128, TT], F32, tag="uni", name="psqk")
                ps = st[m]
                for kc in range(4 * c4, 4 * c4 + 4):
                    nc.tensor.matmul(ps[:], wqk_sb[:, kc, 128 * m:128 * (m + 1)],
                                     xth[kc // 8][:, kc % 8, :],
                                     start=(kc == 0), stop=(kc == KC - 1))
                if c4 == 3:
                    if mode["attn"] or m % 2 == 0:
                        nc.vector.tensor_copy(qraw[:, m, :], ps[:])
                    else:
                        nc.scalar.copy(qraw[:, m, :], ps[:])

            def v_chunk(tc4, c4, st=st):
                if c4 == 0:
                    st[4 + tc4] = unips.tile([128, TT], F32, tag="uni", name="psv")
                psv = st[4 + tc4]
                for kc in range(4 * c4, 4 * c4 + 4):
                    nc.tensor.matmul(psv[:, 0:256],
                                     xth[kc // 8][:, kc % 8, 128 * tc4:128 * (tc4 + 1)],
                                     wv_sb[:, kc, :],
                                     start=(kc == 0), stop=(kc == KC - 1))
                if c4 == 3:
                    if mode["attn"] or tc4 % 2 == 0:
                        nc.vector.tensor_copy(
                            v_sb[:, 4 * j + tc4, :, 0:64],
                            psv[:, 0:256].rearrange("p (h d) -> p h d", h=HL))
                    else:
                        nc.scalar.copy(
                            v_sb[:, 4 * j + tc4, :, 0:64],
                            psv[:, 0:256].rearrange("p (h d) -> p h d", h=HL))

            def rope_bb(bb):
                blk = bb % 2
                src = qraw[:, bb, :]
                dst = qrot[:, blk, :] if bb < 2 else krot[:, blk, tslj]
                psw = unips.tile([128, TT], F32, tag="uni")
                nc.tensor.matmul(psw[:], perm_sb[:], src, start=True, stop=True)
                nc.vector.tensor_tensor(psw[:], psw[:], sin_t[:, blk, :], MUL)
                nc.vector.tensor_tensor(dst, src, cos_t[:, blk, :], MUL)
                nc.vector.tensor_tensor(dst, dst, psw[:], ADD)

            for m in range(4):
                for c4 in range(4):
                    units.append((860, lambda m=m, c4=c4: qk_chunk(m, c4)))
            for tc4 in range(4):
                for c4 in range(4):
                    units.append((440, lambda tc4=tc4, c4=c4: v_chunk(tc4, c4)))
            for bb in range(4):
                units.append((250, lambda bb=bb: rope_bb(bb)))
            return units, qrot

        def make_proj_units(i, ytj, tail_region=False):
            """Partial out rows for t-tile i from its normalized y^T."""
            units = []
            st = {}

            def proj_ct(tc4, ct, st=st):
                if ct == 0:
                    st[tc4] = outp.tile([128, 4, TT], BF16, tag="osb", name="osb")
                osb = st[tc4]
                pso = unips.tile([128, TT], F32, tag="uni")
                for b in range(2):
                    nc.tensor.matmul(pso[:],
                                     ytj[:, b, 128 * tc4:128 * (tc4 + 1)],
                                     wproj_sb[:, b, TT * ct:TT * (ct + 1)],
                                     start=(b == 0), stop=(b == 1))
                if tail_region and ct % 2 == 0:
                    nc.scalar.copy(osb[:, ct, :], pso[:])
                else:
                    nc.vector.tensor_copy(osb[:, ct, :], pso[:])
                if tail_region:
                    # transfer each chunk as soon as it lands, spread across
                    # queues so the issue stream isn't the tail bottleneck
                    eng = (nc.gpsimd, nc.sync, nc.scalar, nc.gpsimd)[tc4]
                    eng.dma_start(
                        out[TT * i + 128 * tc4: TT * i + 128 * (tc4 + 1),
                            TT * ct:TT * (ct + 1)],
                        osb[:, ct, :])
                elif ct == 3:
                    nc.gpsimd.dma_start(
                        out[TT * i + 128 * tc4: TT * i + 128 * (tc4 + 1), :],
                        osb[:].rearrange("p a b -> p (a b)"))

            for tc4 in range(4):
                for ct in range(4):
                    units.append((460, lambda tc4=tc4, ct=ct: proj_ct(tc4, ct)))
            return units

        # ---------- prologue: front(0) runs standalone ----------
        units0, qrot_cur = make_front_units(0)
        for _, fn in units0:
            fn()

        # ---------- main loop ----------
        for i in range(NTT):
            if i + 1 < NTT:
                fu, qrot_next = make_front_units(i + 1)
                qfront.extend(fu)
            else:
                qrot_next = None

            # ---- attention: head PAIRS via tile_position row-tiling ----
            # heads (2bp, 2bp+1) on partitions 0-63 / 64-127; score pair for
            # s-block sb+1 is emitted before av(sb) so the PE never sits on
            # the av->exp dependency; filler drains cover the remaining gap.
            yt = ytp.tile([128, 2, TT], BF16, tag="yt")
            nsb = 4 * (i + 1)
            tails = []

            def score_pair(sb, bp):
                c0 = 128 * (sb - 4 * i) if sb >= 4 * i else 0
                s2 = sps.tile([128, 2 * TT], F32, tag="S")
                nc.tensor.matmul(s2[:, c0:TT],
                                 krot[0:64, bp, 128 * sb:128 * (sb + 1)],
                                 qrot_cur[0:64, bp, c0:TT],
                                 start=True, stop=True, tile_position=(0, 0))
                nc.tensor.matmul(s2[:, TT + c0:2 * TT],
                                 krot[64:128, bp, 128 * sb:128 * (sb + 1)],
                                 qrot_cur[64:128, bp, c0:TT],
                                 start=True, stop=True, tile_position=(64, 0))
                return s2, c0

            for bp in range(2):
                psyA = psyp.tile([65, TT], F32, tag="psy")
                psyB = psyp.tile([65, TT], F32, tag="psy")
                cur = score_pair(0, bp)
                for sb in range(nsb):
                    s2, c0 = cur
                    p4 = p4p.tile([128, 2 * TT], BF16, tag="P4")
                    if c0 == 0:
                        nc.scalar.activation(p4[:], s2[:], EXP, scale=0.125)
                    else:
                        # diagonal block: per-head contiguous slices (grouped
                        # strided APs mis-lower on the activation engine)
                        for hh in range(2):
                            o = TT * hh
                            nc.scalar.activation(p4[:, o + c0:o + TT],
                                                 s2[:, o + c0:o + TT],
                                                 EXP, scale=0.125)
                    if sb >= 4 * i:
                        # exact-diagonal [128,128] band keep-mask per head
                        for hh in range(2):
                            o = TT * hh
                            nc.gpsimd.tensor_tensor(
                                p4[:, o + c0:o + c0 + 128],
                                p4[:, o + c0:o + c0 + 128],
                                tri_sb[:], MUL)
                    if sb + 1 < nsb:
                        cur = score_pair(sb + 1, bp)
                    drain(500)
                    nc.tensor.matmul(psyA[:, c0:TT], v_sb[:, sb, 2 * bp, 0:65],
                                     p4[:, c0:TT],
                                     start=(sb == 0), stop=(sb == nsb - 1),
                                     skip_group_check=True)
                    nc.tensor.matmul(psyB[:, c0:TT], v_sb[:, sb, 2 * bp + 1, 0:65],
                                     p4[:, TT + c0:2 * TT],
                                     start=(sb == 0), stop=(sb == nsb - 1),
                                     skip_group_check=True)
                # evacuate psy (incl. denominator row) so the psy slot frees,
                # and kick off the reciprocal; broadcast+normalize deferred
                for hh, psy in ((0, psyA), (1, psyB)):
                    ym65 = ymp.tile([65, TT], F32, tag="ym")
                    nc.scalar.copy(ym65[:], psy[:])
                    den0 = rp.tile([1, TT], F32, tag=f"d{bp}{hh}")
                    nc.vector.tensor_copy(den0[:], ym65[64:65, :])
                    rsb = rp.tile([1, TT], F32, tag=f"r{bp}{hh}")
                    nc.vector.reciprocal_approx_fast(out=rsb[:], in_=den0[:])
                    rsb2 = rp.tile([1, TT], BF16, tag=f"rb{bp}{hh}")
                    nc.vector.tensor_copy(rsb2[:], rsb[:])
                    tails.append((bp, hh, ym65, rsb2))
                if bp == 0:
                    drain(1200)

            for ti, (bp, hh, ym65, rsb) in enumerate(tails):
                if ti == 2:
                    # bp1's reciprocal chain is still in flight; keep the PE
                    # fed while it completes
                    drain(3000)
                psb = unips.tile([128, TT], F32, tag="uni")
                nc.tensor.matmul(psb[0:64, :], ones_sb[:],
                                 rsb[:], start=True, stop=True)
                if hh == 0:
                    dst = yt[0:64, bp, :]
                else:
                    ytm = ytmpp.tile([64, TT], BF16, tag="ytmp2")
                    dst = ytm[:]
                nc.vector.tensor_tensor(dst, ym65[0:64, :], psb[0:64, :], MUL)
                if hh != 0:
                    nc.scalar.dma_start(yt[64:128, bp, :], dst)

            # front(i+1) must be complete before attention(i+1)
            drain_front_all()
            qrot_cur = qrot_next

            if i + 1 < NTT:
                qproj.extend(make_proj_units(i, yt))
            else:
                drain_all()
                for _, fn in make_proj_units(i, yt, tail_region=True):
                    fn()

    nc.finalize()
    return nc


def _host_inputs(x, w_qkv, w_proj, attn_mask):
    """Build the 8 per-core input maps (host-side sharding/layout prep)."""
    import ml_dtypes
    BF = ml_dtypes.bfloat16
    x = np.asarray(x)
    w_qkv = np.asarray(w_qkv)
    w_proj = np.asarray(w_proj)
    attn_mask = np.asarray(attn_mask)

    xT = np.ascontiguousarray(x.reshape(T, C).T).astype(BF)

    # RoPE tables, faithful to the reference broadcasting quirk:
    # head g rotates all pairs by angle t * theta^(-g/32) (f32 math).
    inv_freq = (1.0 / (ROPE_THETA ** (np.arange(0, D, 2, dtype=np.float32) / D))
                ).astype(np.float32)                     # [32] indexed by head
    t_ar = np.arange(T, dtype=np.float32)
    freqs = (t_ar[:, None] * inv_freq[None, :]).astype(np.float32)  # [T, 32]
    cosf = np.cos(freqs).astype(np.float32)              # [T, 32]
    sinf = np.sin(freqs).astype(np.float32)
    sgn = np.where(np.arange(64) % 2 == 0, np.float32(-1.0), np.float32(1.0))  # [64]

    # 0/1 keep-mask for the exact-diagonal [128,128] band, from the actual
    # mask: band element (p, tb) keeps iff attn_mask[tb, p] == 0
    trib = np.ascontiguousarray(
        np.exp(attn_mask[0:128, 0:128].astype(np.float64)).T).astype(BF)

    permM = np.zeros((128, 128), dtype=np.float32)
    permM[np.arange(128), np.arange(128) ^ 1] = 1.0
    permM = permM.astype(BF)

    in_maps = []
    for c in range(NC_):
        wqk_c = np.ascontiguousarray(np.concatenate(
            [w_qkv[:, 256 * c:256 * (c + 1)],
             w_qkv[:, 2048 + 256 * c:2048 + 256 * (c + 1)]], axis=1)).astype(BF)
        wv_c = np.ascontiguousarray(
            w_qkv[:, 4096 + 256 * c:4096 + 256 * (c + 1)]).astype(BF)
        wproj_c = np.ascontiguousarray(w_proj[256 * c:256 * (c + 1), :]).astype(BF)

        costab = np.empty((128, 2, T), dtype=np.float32)
        sintab = np.empty((128, 2, T), dtype=np.float32)
        for bb in range(2):
            for p in range(128):
                g = 4 * c + 2 * bb + (p // 64)           # global head
                costab[p, bb, :] = cosf[:, g]
                sintab[p, bb, :] = sgn[p % 64] * sinf[:, g]

        in_maps.append({
            "xt": xT, "wqk": wqk_c, "wv": wv_c, "wproj": wproj_c,
            "costab": costab.astype(BF), "sintab": sintab.astype(BF),
            "tri": trib, "perm": permM,
        })
    return in_maps


def _get_program():
    if "nc" not in _CACHE:
        _CACHE["nc"] = _build_program()
    return _CACHE["nc"]


def run_sharded(in_maps, trace=False):
    from concourse.bass_utils import run_bass_kernel_spmd
    nc = _get_program()
    return run_bass_kernel_spmd(nc, in_maps, list(range(NC_)), trace=trace)


def kernel(x, w_qkv, w_proj, attn_mask):
    in_maps = _host_inputs(x, w_qkv, w_proj, attn_mask)
    res = run_sharded(in_maps)
    acc = res.results[0]["out"].astype(np.float32).copy()
    for c in range(1, NC_):
        acc += res.results[c]["out"].astype(np.float32)
    return acc.reshape(1, T, C)
